# revision 11
# baseline (speedup 1.0000x reference)
"""GQA attention prefill kernel for Trainium2, 8-way (batch x kv-head-group) sharded.

Problem shapes (hardcoded): B=4, S=2048, HID=1536, H=12 q heads, HKV=2 kv heads,
D=128. Each of the 8 cores handles one (batch b, kv-group g) pair: 6 q heads +
their kv head, full sequence. Wqkv rows / Wo columns are sharded by head; the
pairwise Wo partial sums are reduced on the host during unsharding.

Device-side dataflow is fully "transposed" (feature dim on partitions) so no
on-chip transposes are needed except 16 PE transposes of V:
  qkvT[m, s]   = WqkvT.T @ xT        (m = 6*128 q | 128 k | 128 v)
  RoPE on q,k  (DVE, sign-folded sin table)
  scoresT      = kT.T @ qT           [sk, sq] tiles, causal tiles only
  attnT        = exp(scoresT/sqrt(D)) * mask
  den          = ones.T @ attnT      (softmax denominator, per sq)
  outT         = v_sd.T @ attnT      [d, sq]
  proj         = outT_norm.T @ WoT   [s, n] partial, host sums the g pair
"""

import numpy as np

_MAX_WAITS = 1  # walrus in this image: 1 sync wait per CTRL instruction


def _install_drain_split():
    import concourse.tile as tile_mod
    import bass_rust

    if getattr(tile_mod.TileContext, "_drain_split_patched", False):
        return

    def _drain_and_barrier(self, tick_clock, wait_clock):
        from concourse.vector_clock import ScopedClock

        nc = self.nc
        drain_inst = nc.sync.drain()
        wait_clock.add_sem_waits(
            drain_inst.ins, ScopedClock({None: tick_clock.global_clock})
        )
        mi = drain_inst.ins
        si = mi.sync_info
        if si is not None and si.on_wait and len(si.on_wait) > _MAX_WAITS:
            waits = list(si.on_wait)
            si.on_wait = waits[:_MAX_WAITS]
            mi.sync_info = si
            for i in range(_MAX_WAITS, len(waits), _MAX_WAITS):
                d2 = nc.sync.drain()
                d2.ins.sync_info = bass_rust.SyncInfo(
                    on_wait=waits[i : i + _MAX_WAITS], on_update=[]
                )

        nc.all_engine_barrier()
        assert self.sems is not None
        popped = nc._tile_sem_poison_stack.pop()
        assert popped is self._sem_poison
        nc.clear_and_free_semaphores(list(self.sems.allocated().values()))
        nc.all_engine_barrier()

    tile_mod.TileContext._drain_and_barrier = _drain_and_barrier
    tile_mod.TileContext._drain_split_patched = True


def _split_waits_in_bir(bir_json: bytes) -> bytes:
    """Walrus here accepts a single sync wait per instruction; hoist the
    excess onto injected same-engine Nops placed just before the owner."""
    import json

    d = json.loads(bir_json)
    changed = False
    for fn in d.get("functions", []):
        for bb in fn.get("blocks", []):
            out = []
            for inst in bb["instructions"]:
                si = inst.get("sync_info") or {}
                ow = si.get("on_wait") or []
                if len(ow) > _MAX_WAITS:
                    changed = True
                    keep = ow[-_MAX_WAITS:]
                    extra = ow[:-_MAX_WAITS]
                    for i, w in enumerate(extra):
                        nop = {
                            "engine": inst["engine"],
                            "ins": [],
                            "name": f"{inst['name']}ws{i}",
                            "opcode": "NoOp",
                            "outs": [],
                            "sync_info": {"on_update": [], "on_wait": [w]},
                        }
                        if "debug" in inst:
                            nop["debug"] = inst["debug"]
                        out.append(nop)
                    si["on_wait"] = keep
                out.append(inst)
            bb["instructions"] = out
    if not changed:
        return bir_json
    return json.dumps(d).encode()


def _install_wait_split():
    import concourse.bass_utils as bu
    import concourse.bass2jax as b2j

    if getattr(bu, "_wait_split_patched", False):
        return
    orig = bu.compile_bir_kernel

    def patched(bir_json, tmpdir, neff_name="file.neff"):
        return orig(_split_waits_in_bir(bytes(bir_json)), tmpdir, neff_name)

    bu.compile_bir_kernel = patched
    b2j.compile_bir_kernel = patched
    bu._wait_split_patched = True


H, HKV, D, HID = 12, 2, 128, 1536
B, S = 4, 2048
NSLOTS = 16384
QH = H // HKV  # q heads per group: 6
MROWS = QH * D + 2 * D  # 1024 Wqkv rows per core
SCALE = 1.0 / float(np.sqrt(D))

SQ_CHUNK = 512
N_QC = S // SQ_CHUNK  # 4
N_KB = S // 128  # 16
KB_PER_QC = SQ_CHUNK // 128  # 4

_nc_cache = {}


def _build_nc():
    import concourse.bass as bass
    import concourse.mybir as mybir
    import concourse.tile as tile
    from concourse.bass import ds, ts
    from contextlib import ExitStack

    _install_drain_split()
    _install_wait_split()

    f32 = mybir.dt.float32
    f32r = mybir.dt.float32r
    AF = mybir.ActivationFunctionType

    nc = bass.Bass()

    xT = nc.dram_tensor("xT", [HID, S], f32r, kind="ExternalInput")
    wqkvT = nc.dram_tensor("wqkvT", [HID, MROWS], f32r, kind="ExternalInput")
    bias = nc.dram_tensor("bias", [128, 8], f32, kind="ExternalInput")
    woT = nc.dram_tensor("woT", [QH * D, HID], f32r, kind="ExternalInput")
    cosT = nc.dram_tensor("cosT", [D, S], f32, kind="ExternalInput")
    sinT = nc.dram_tensor("sinT", [D, S], f32, kind="ExternalInput")
    masks = nc.dram_tensor("masks", [KB_PER_QC, 128, SQ_CHUNK], f32,
                           kind="ExternalInput")
    ident_in = nc.dram_tensor("ident", [128, 128], f32, kind="ExternalInput")
    ones_in = nc.dram_tensor("ones", [128, 1], f32r, kind="ExternalInput")

    proj = nc.dram_tensor("proj", [S, HID], f32, kind="ExternalOutput")
    k_out = nc.dram_tensor("k_out", [D, S], f32, kind="ExternalOutput")
    v_out = nc.dram_tensor("v_out", [S, D], f32, kind="ExternalOutput")

    def r(ap):
        return ap.bitcast(f32r)

    with tile.TileContext(nc) as tc, ExitStack() as ctx:
        # ---- persistent pools -------------------------------------------
        const = ctx.enter_context(tc.tile_pool(name="const", bufs=1))
        cosT_sb = const.tile([D, S], f32)
        nc.sync.dma_start(out=cosT_sb, in_=cosT[:, :])
        sinT_sb = const.tile([D, S], f32)
        nc.sync.dma_start(out=sinT_sb, in_=sinT[:, :])
        masks_sb = const.tile([128, KB_PER_QC, SQ_CHUNK], f32)
        nc.sync.dma_start(out=masks_sb, in_=masks.rearrange("j p f -> p j f"))
        bias_sb = const.tile([128, 8], f32)
        nc.sync.dma_start(out=bias_sb, in_=bias[:, :])
        ident = const.tile([128, 128], f32)
        nc.sync.dma_start(out=ident, in_=ident_in[:, :])
        ones_col = const.tile([128, 1], f32r)
        nc.sync.dma_start(out=ones_col, in_=ones_in[:, :])

        qkvp = ctx.enter_context(tc.tile_pool(name="qkvp", bufs=1))
        qkvT = qkvp.tile([128, 7, S], f32r)  # blocks: q0..q5, k (pre-rounded)
        vp = ctx.enter_context(tc.tile_pool(name="vp", bufs=1))
        v_sdr = vp.tile([128, N_KB, D], f32r)
        vtp = tc.alloc_tile_pool(name="vtp", bufs=1)
        vT = vtp.tile([128, S], f32)

        # ---- stage 1: qkv projection ------------------------------------
        xT_r = xT.rearrange("(kb p) s -> p kb s", p=128)
        wq_r = wqkvT.rearrange("(kb p) m -> p kb m", p=128)
        with tc.tile_pool(name="s1", bufs=1) as s1, \
             tc.tile_pool(name="ps1", bufs=1, space="PSUM") as ps1:
            wq = s1.tile([128, HID // 128, MROWS], f32r)
            nc.sync.dma_start(out=wq, in_=wq_r)
            for sc in range(N_QC):
                xt = s1.tile([128, HID // 128, SQ_CHUNK], f32r, tag="xt", bufs=2)
                nc.sync.dma_start(out=xt, in_=xT_r[:, :, ds(sc * SQ_CHUNK, SQ_CHUNK)])
                for m in range(8):
                    ps = ps1.tile([128, SQ_CHUNK], f32, tag="s1ps", bufs=4)
                    for kb in range(HID // 128):
                        nc.tensor.matmul(
                            ps,
                            lhsT=wq[:, kb, ts(m, 128)],
                            rhs=xt[:, kb, :],
                            start=(kb == 0),
                            stop=(kb == HID // 128 - 1),
                        )
                    dst = (qkvT[:, m, ds(sc * SQ_CHUNK, SQ_CHUNK)] if m < 7
                           else vT[:, ds(sc * SQ_CHUNK, SQ_CHUNK)])
                    nc.vector.tensor_scalar_add(
                        out=dst,
                        in0=ps,
                        scalar1=bias_sb[:, m : m + 1],
                    )

        # ---- RoPE on q (blocks 0..5) and k (block 6), in place ----------
        with tc.tile_pool(name="rope", bufs=1) as rp:
            rot6 = rp.tile([128, QH, S], f32)
            nc.vector.tensor_copy(out=rot6[0:64, :, :], in_=qkvT[64:128, 0:QH, :].bitcast(f32))
            nc.vector.tensor_copy(out=rot6[64:128, :, :], in_=qkvT[0:64, 0:QH, :].bitcast(f32))
            nc.vector.tensor_mul(
                out=rot6,
                in0=rot6,
                in1=sinT_sb[:, None, :].to_broadcast([128, QH, S]),
            )
            nc.vector.tensor_mul(
                out=qkvT[:, 0:QH, :],
                in0=qkvT[:, 0:QH, :].bitcast(f32),
                in1=cosT_sb[:, None, :].to_broadcast([128, QH, S]),
            )
            nc.vector.tensor_add(out=qkvT[:, 0:QH, :], in0=qkvT[:, 0:QH, :].bitcast(f32), in1=rot6)

            rot1 = rp.tile([128, S], f32)
            nc.vector.tensor_copy(out=rot1[0:64, :], in_=qkvT[64:128, 6, :].bitcast(f32))
            nc.vector.tensor_copy(out=rot1[64:128, :], in_=qkvT[0:64, 6, :].bitcast(f32))
            nc.vector.tensor_mul(out=rot1, in0=rot1, in1=sinT_sb)
            nc.vector.tensor_mul(out=qkvT[:, 6, :], in0=qkvT[:, 6, :].bitcast(f32), in1=cosT_sb)
            nc.vector.tensor_add(out=qkvT[:, 6, :], in0=qkvT[:, 6, :].bitcast(f32), in1=rot1)

        # k cache output (RoPE'd kT)
        nc.sync.dma_start(out=k_out[:, :], in_=qkvT[:, 6, :].bitcast(f32))

        # ---- V transpose: vT [d, s] -> v_sd [s-block, kb, d] ------------
        with tc.tile_pool(name="vsdp", bufs=1) as vsdp, \
             tc.tile_pool(name="psv", bufs=2, space="PSUM") as psv:
            v_sd = vsdp.tile([128, N_KB, D], f32)
            for kb in range(N_KB):
                pvt = psv.tile([128, 128], f32, tag="vt")
                nc.tensor.transpose(pvt, vT[:, ts(kb, 128)], ident)
                nc.vector.tensor_copy(out=v_sd[:, kb, :], in_=pvt)
                nc.vector.tensor_copy(out=v_sdr[:, kb, :], in_=pvt)
            nc.sync.dma_start(out=v_out.rearrange("(kb p) d -> p kb d", p=128),
                              in_=v_sd)
        vtp.release()

        # ---- attention + Wo ---------------------------------------------
        outp = ctx.enter_context(tc.tile_pool(name="outp", bufs=1))
        outT = outp.tile([128, QH, S], f32r)  # normalized attention out, [d, h, s]
        wop = ctx.enter_context(tc.tile_pool(name="wop", bufs=1))
        wo_sb = wop.tile([128, QH, HID], f32r)
        nc.sync.dma_start(out=wo_sb, in_=woT.rearrange("(h p) n -> p h n", p=128))

        atp = ctx.enter_context(tc.tile_pool(name="atp", bufs=1))
        denp = ctx.enter_context(tc.tile_pool(name="denp", bufs=1))
        dram = ctx.enter_context(tc.tile_pool(name="dram", bufs=2, space="DRAM"))
        stp = ctx.enter_context(tc.tile_pool(name="stp", bufs=1))
        psA = ctx.enter_context(tc.tile_pool(name="psA", bufs=1, space="PSUM"))

        for h in range(QH):
            for qc in range(N_QC):
                nkb = KB_PER_QC * (qc + 1)
                ps_pv = psA.tile([128, SQ_CHUNK], f32, tag="pv", bufs=2)
                ps_den = psA.tile([1, SQ_CHUNK], f32, tag="den", bufs=2)
                for kb in range(nkb):
                    ps_s = psA.tile([128, SQ_CHUNK], f32, tag="sc", bufs=2)
                    nc.tensor.matmul(
                        ps_s,
                        lhsT=qkvT[:, 6, ts(kb, 128)],
                        rhs=qkvT[:, h, ds(qc * SQ_CHUNK, SQ_CHUNK)],
                        start=True,
                        stop=True,
                    )
                    at = atp.tile([128, SQ_CHUNK], f32r, tag="at", bufs=3)
                    nc.scalar.activation(out=at, in_=ps_s, func=AF.Exp, scale=SCALE)
                    j = kb - KB_PER_QC * qc
                    if j >= 0:  # diagonal tile: causal mask
                        nc.vector.tensor_mul(out=at, in0=at.bitcast(f32), in1=masks_sb[:, j, :])
                    nc.tensor.matmul(
                        ps_den,
                        lhsT=ones_col,
                        rhs=at,
                        start=(kb == 0),
                        stop=(kb == nkb - 1),
                    )
                    nc.tensor.matmul(
                        ps_pv,
                        lhsT=v_sdr[:, kb, :],
                        rhs=at,
                        start=(kb == 0),
                        stop=(kb == nkb - 1),
                    )
                # normalize: outT[:, h, chunk] = ps_pv * (1/den) broadcast
                rec = denp.tile([1, SQ_CHUNK], f32, tag="rec", bufs=2)
                nc.vector.reciprocal(out=rec, in_=ps_den)
                rec_dr = dram.tile([1, SQ_CHUNK], f32, tag="rdr", bufs=2)
                nc.sync.dma_start(out=rec_dr, in_=rec)
                rec_bc = denp.tile([128, SQ_CHUNK], f32, tag="rbc", bufs=2)
                nc.gpsimd.dma_start(
                    out=rec_bc, in_=rec_dr.to_broadcast([128, SQ_CHUNK])
                )
                nc.vector.tensor_mul(
                    out=outT[:, h, ds(qc * SQ_CHUNK, SQ_CHUNK)],
                    in0=ps_pv,
                    in1=rec_bc,
                )

        # ---- Wo projection (partial; host sums the pair) ----------------
        for sb in range(S // 128):
            for nch in range(HID // SQ_CHUNK):
                ps_p = psA.tile([128, SQ_CHUNK], f32, tag="proj", bufs=2)
                for h in range(QH):
                    nc.tensor.matmul(
                        ps_p,
                        lhsT=outT[:, h, ts(sb, 128)],
                        rhs=wo_sb[:, h, ds(nch * SQ_CHUNK, SQ_CHUNK)],
                        start=(h == 0),
                        stop=(h == QH - 1),
                    )
                stg = stp.tile([128, SQ_CHUNK], f32, tag="stg", bufs=2)
                nc.vector.tensor_copy(out=stg, in_=ps_p)
                nc.sync.dma_start(
                    out=proj[ts(sb, 128), ds(nch * SQ_CHUNK, SQ_CHUNK)], in_=stg
                )

    return nc


def _get_nc():
    if "nc" not in _nc_cache:
        _nc_cache["nc"] = _build_nc()
    return _nc_cache["nc"]


def _round_f32r(a):
    u = np.ascontiguousarray(a, dtype=np.float32).view(np.uint32).astype(np.uint64)
    r = ((u + 0x7FF + ((u >> 12) & 1)) & 0xFFFFF000).astype(np.uint32)
    return r.view(np.float32)


def kernel(x, cos, sin, kv_cache_k, kv_cache_v, slot_mapping, Wqkv, bqkv, Wo,
           is_prefill):
    from concourse.bass_utils import run_bass_kernel_spmd

    x = np.asarray(x, dtype=np.float32)
    cos = np.asarray(cos, dtype=np.float32)
    sin = np.asarray(sin, dtype=np.float32)
    kv_cache_k = np.asarray(kv_cache_k, dtype=np.float32)
    kv_cache_v = np.asarray(kv_cache_v, dtype=np.float32)
    slot_mapping = np.asarray(slot_mapping)
    Wqkv = np.asarray(Wqkv, dtype=np.float32)
    bqkv = np.asarray(bqkv, dtype=np.float32)
    Wo = np.asarray(Wo, dtype=np.float32)

    nc = _get_nc()

    q_size = H * D  # 1536
    ident = np.eye(128, dtype=np.float32)
    mask = np.zeros((KB_PER_QC, 128, SQ_CHUNK), dtype=np.float32)
    pp, ff = np.arange(128)[:, None], np.arange(SQ_CHUNK)[None, :]
    for j in range(KB_PER_QC):
        mask[j] = (ff >= pp + 128 * j).astype(np.float32)

    in_maps = []
    for c in range(8):
        b, g = divmod(c, 2)
        # Wqkv row shard: q heads 6g..6g+5, k head g, v head g
        q_rows = slice(g * QH * D, (g + 1) * QH * D)
        k_rows = slice(q_size + g * D, q_size + (g + 1) * D)
        v_rows = slice(q_size + HKV * D + g * D, q_size + HKV * D + (g + 1) * D)
        w_shard = np.concatenate([Wqkv[q_rows], Wqkv[k_rows], Wqkv[v_rows]], axis=0)
        b_shard = np.concatenate([bqkv[q_rows], bqkv[k_rows], bqkv[v_rows]], axis=0)

        sin_signed = sin[b].copy()  # (S, D)
        sin_signed[:, : D // 2] *= -1.0

        in_maps.append({
            "xT": _round_f32r(np.ascontiguousarray(x[b].T)),
            "wqkvT": _round_f32r(np.ascontiguousarray(w_shard.T)),
            "bias": np.ascontiguousarray(b_shard.reshape(8, 128).T),
            "woT": _round_f32r(np.ascontiguousarray(Wo[:, g * QH * D : (g + 1) * QH * D].T)),
            "cosT": np.ascontiguousarray(cos[b].T),
            "sinT": np.ascontiguousarray(sin_signed.T),
            "masks": mask,
            "ident": ident,
            "ones": np.ones((128, 1), dtype=np.float32),
        })

    res = run_bass_kernel_spmd(nc, in_maps, core_ids=list(range(8)))
    results = res.results

    out = np.empty((B, S, HID), dtype=np.float32)
    k_flat = np.zeros((B * S, HKV, D), dtype=np.float32)
    v_flat = np.zeros((B * S, HKV, D), dtype=np.float32)
    for c in range(8):
        b, g = divmod(c, 2)
        if g == 0:
            out[b] = results[c]["proj"]
        else:
            out[b] += results[c]["proj"]
        k_flat[b * S : (b + 1) * S, g, :] = results[c]["k_out"].T
        v_flat[b * S : (b + 1) * S, g, :] = results[c]["v_out"]

    kc = kv_cache_k.copy()
    vc = kv_cache_v.copy()
    kc[slot_mapping] = k_flat
    vc[slot_mapping] = v_flat
    return out, kc, vc
